# revision 4
# baseline (speedup 1.0000x reference)
"""TRN2 Bass kernel for nn_BrainModule (sparse_attention).

Computation (per sample b):
  emb[c,d]   = fourier embedding of positions[b,c]          (d = 242)
  scores[o,c]= heads[subj[b]][o,:] . emb[c,:] + offset[c]   (offset = -1e9 on
                                                             invalid channels)
  w[o,c]     = softmax_c(scores)
  out[o,t]   = sum_c w[o,c] * meg[b,c,t]

Strategy: data-parallel over batch B=32 across 8 cores (4 samples each).
On device, everything is computed in the [C, O] ("transposed") orientation so
the big einsum consumes the softmax weights directly as the matmul stationary
operand, with the 1/sum normalization folded into the PSUM->SBUF copy:

  xsT[k,c]    = a[c]*fi[k] + b[c]*fj[k]      (DVE outer product; a,b host-scaled
                                              positions, fi/fj integer freqs)
  frac        = xsT - round(xsT)             (int32-cast round trick: HW Sin
                                              only valid on small args)
  embB = sin(2*pi*frac), embA = cos via sin(2*pi*frac(xsT+0.25))  [121, C]
  embA row 121 = offset[c], hTA row 121 = 1  (folds the invalid-channel -1e9
                                              into the score matmul)
  scoresT[c,o] (psum) = embA_chunk.T @ hTA + embB_chunk.T @ hTB
  wT_un[c,o]  = exp(scoresT)                 (f32r; invalid channels -> exactly 0)
  sums[o]     = ones.T-matmul over c of wT_un; inv = 1/sums (DVE)
  out[o,t]    = (sum_c wT_un[c,o]*meg[c,t]) * inv[o]   (fp32r matmuls, N=512;
                                                        scale during psum copy)

All matmuls use float32r (1 cycle/row at N>=256 vs 4 for fp32; measured HW
rel-err ~2e-4, same as the fp32 path on this silicon).
"""
import numpy as np

B, C, T = 32, 273, 4096
CHOUT = 270
N_FREQS = 11
NF2 = N_FREQS * N_FREQS          # 121
D_A = NF2 + 1                    # cos half + offset/ones row
MARGIN = 0.2
WIDTH = 1.0 + 2.0 * MARGIN
INVALID = -0.1
NEG_INF = -1e9
N_CORES = 8
BS = B // N_CORES                # samples per core
TWO_PI = float(2.0 * np.pi)
# largest f32 <= 2*pi, so |frac| = 0.5 never maps beyond pi
SCALE_2PI = float(np.nextafter(np.float32(2.0 * np.pi), np.float32(0.0)))

C_CHUNKS = [(0, 128), (128, 128), (256, C - 256)]      # K chunks over channels
M_CHUNKS = [(0, 128), (128, 128), (256, CHOUT - 256)]  # partition chunks over O
TH = 2048                                              # t-half size
NT_Q = TH // 512                                       # 512-wide psum tiles/half

_NC_CACHE = {}


def _build_bass():
    import concourse.bacc as bacc
    import concourse.mybir as mybir
    import concourse.tile as tile

    F32 = mybir.dt.float32
    F32R = mybir.dt.float32r
    I32 = mybir.dt.int32
    Sin = mybir.ActivationFunctionType.Sin
    Exp = mybir.ActivationFunctionType.Exp

    nc = bacc.Bacc("TRN2", target_bir_lowering=False, debug=False,
                   num_devices=N_CORES)

    meg_d = nc.dram_tensor("meg", [BS, C, T], F32R, kind="ExternalInput")
    pa_d = nc.dram_tensor("pa", [BS, C], F32, kind="ExternalInput")
    pb_d = nc.dram_tensor("pb", [BS, C], F32, kind="ExternalInput")
    offs_d = nc.dram_tensor("offs", [BS, C], F32R, kind="ExternalInput")
    hta_d = nc.dram_tensor("hta", [BS, D_A, CHOUT], F32R, kind="ExternalInput")
    htb_d = nc.dram_tensor("htb", [BS, NF2, CHOUT], F32R, kind="ExternalInput")
    fi_d = nc.dram_tensor("fi", [NF2, 1], F32, kind="ExternalInput")
    fj_d = nc.dram_tensor("fj", [NF2, 1], F32, kind="ExternalInput")
    ones_d = nc.dram_tensor("ones", [128, 1], F32, kind="ExternalInput")
    out_d = nc.dram_tensor("out", [BS, CHOUT, T], F32, kind="ExternalOutput")

    import concourse.bass as bass

    with tile.TileContext(nc) as tc:
        with (
            tc.tile_pool(name="const", bufs=1) as const,
            tc.tile_pool(name="wsb", bufs=2) as wsb,
            tc.tile_pool(name="megp", bufs=2) as megp,
            tc.tile_pool(name="outp", bufs=2) as outp,
            tc.tile_pool(name="wps", bufs=2, space="PSUM") as wps,
            tc.tile_pool(name="bps", bufs=4, space="PSUM") as bps,
        ):
            fi = const.tile([NF2, 1], F32, tag="fi")
            fj = const.tile([NF2, 1], F32, tag="fj")
            ones = const.tile([128, 1], F32, tag="ones")
            nc.sync.dma_start(out=fi, in_=fi_d[:, :])
            nc.sync.dma_start(out=fj, in_=fj_d[:, :])
            nc.sync.dma_start(out=ones, in_=ones_d[:, :])

            for b in range(BS):
                # ---- weight stage -------------------------------------
                a_rep = wsb.tile([NF2, C], F32, tag="a_rep")
                b_rep = wsb.tile([NF2, C], F32, tag="b_rep")
                pa_bcast = bass.AP(
                    tensor=pa_d, offset=b * C, ap=[[0, NF2], [1, C]])
                pb_bcast = bass.AP(
                    tensor=pb_d, offset=b * C, ap=[[0, NF2], [1, C]])
                nc.gpsimd.dma_start(out=a_rep, in_=pa_bcast)
                nc.gpsimd.dma_start(out=b_rep, in_=pb_bcast)

                hta = wsb.tile([D_A, CHOUT], F32R, tag="hta")
                htb = wsb.tile([NF2, CHOUT], F32R, tag="htb")
                nc.sync.dma_start(out=hta, in_=hta_d[b, :, :])
                nc.sync.dma_start(out=htb, in_=htb_d[b, :, :])

                # xsT = a*fi + b*fj   [121, C]
                xs = wsb.tile([NF2, C], F32, tag="xs")
                nc.vector.tensor_scalar_mul(out=xs, in0=a_rep, scalar1=fi)
                xs2 = wsb.tile([NF2, C], F32, tag="xs2")
                nc.vector.tensor_scalar_mul(out=xs2, in0=b_rep, scalar1=fj)
                nc.vector.tensor_add(out=xs, in0=xs, in1=xs2)

                embA = wsb.tile([D_A, C], F32R, tag="embA")
                embB = wsb.tile([NF2, C], F32R, tag="embB")

                # two-stage range reduction to [-0.5, 0.5]; robust to the
                # f32->int32 cast semantics (HW rounds, CoreSim truncates)
                ki = wsb.tile([NF2, C], I32, tag="ki")
                kf = wsb.tile([NF2, C], F32, tag="kf")
                frac = wsb.tile([NF2, C], F32, tag="frac")

                def reduce_frac(src):
                    # frac1 = src - cast(src)  in (-1, 1)
                    nc.vector.tensor_copy(ki, src)
                    nc.vector.tensor_copy(kf, ki)
                    nc.vector.tensor_sub(out=frac, in0=src, in1=kf)
                    # wrap into [-0.5, 0.5]: frac -= (frac > 0.5); frac += (frac < -0.5)
                    nc.vector.tensor_scalar(out=kf, in0=frac, scalar1=0.5,
                                            scalar2=None,
                                            op0=mybir.AluOpType.is_gt)
                    nc.vector.tensor_sub(out=frac, in0=frac, in1=kf)
                    nc.vector.tensor_scalar(out=kf, in0=frac, scalar1=-0.5,
                                            scalar2=None,
                                            op0=mybir.AluOpType.is_lt)
                    nc.vector.tensor_add(out=frac, in0=frac, in1=kf)
                    return frac

                # sin half
                reduce_frac(xs)
                nc.scalar.activation(out=embB, in_=frac, func=Sin,
                                     scale=SCALE_2PI)
                # cos half: cos(2pi x) = sin(2pi (x + 0.25))
                nc.vector.tensor_scalar_add(out=xs2, in0=xs, scalar1=0.25)
                reduce_frac(xs2)
                nc.scalar.activation(out=embA[0:NF2, :], in_=frac, func=Sin,
                                     scale=SCALE_2PI)
                # offset row
                nc.sync.dma_start(out=embA[NF2:D_A, :], in_=offs_d[b:b + 1, :])

                # scoresT chunks + exp -> unnormalized transposed weights
                wt = []
                for ci, (c0, cs) in enumerate(C_CHUNKS):
                    ps_s = wps.tile([128, CHOUT], F32, tag="ps_s")
                    nc.tensor.matmul(ps_s[0:cs, :], embA[:, c0:c0 + cs], hta,
                                     start=True, stop=False)
                    nc.tensor.matmul(ps_s[0:cs, :], embB[:, c0:c0 + cs], htb,
                                     start=False, stop=True)
                    w_un = wsb.tile([128, CHOUT], F32R, tag=f"w_un{ci}")
                    nc.scalar.activation(out=w_un[0:cs, :], in_=ps_s[0:cs, :],
                                         func=Exp)
                    wt.append(w_un)

                # per-output-channel 1/sum via ones-matmul over the c chunks
                invs = []
                for mi, (m0, ms) in enumerate(M_CHUNKS):
                    ps_sum = wps.tile([128, 1], F32, tag="ps_sum")
                    for ci, (c0, cs) in enumerate(C_CHUNKS):
                        nc.tensor.matmul(ps_sum[0:ms, :],
                                         wt[ci][0:cs, m0:m0 + ms].bitcast(F32),
                                         ones[0:cs, :],
                                         start=(ci == 0), stop=(ci == 2))
                    inv = wsb.tile([128, 1], F32, tag=f"inv{mi}")
                    nc.vector.reciprocal(out=inv[0:ms, :], in_=ps_sum[0:ms, :])
                    invs.append(inv)

                # ---- big matmul stage ---------------------------------
                for th in range(T // TH):
                    t0 = th * TH
                    megs = []
                    for ci, (c0, cs) in enumerate(C_CHUNKS):
                        mg = megp.tile([cs, TH], F32R, tag=f"mg{ci}")
                        nc.sync.dma_start(
                            out=mg, in_=meg_d[b, c0:c0 + cs, t0:t0 + TH])
                        megs.append(mg)
                    for mi, (m0, ms) in enumerate(M_CHUNKS):
                        ot = outp.tile([ms, TH], F32, tag=f"ot{mi}")
                        for tq in range(NT_Q):
                            ps_o = bps.tile([128, 512], F32, tag="ps_o")
                            for ci, (c0, cs) in enumerate(C_CHUNKS):
                                nc.tensor.matmul(
                                    ps_o[0:ms, :],
                                    wt[ci][0:cs, m0:m0 + ms],
                                    megs[ci][:, tq * 512:(tq + 1) * 512],
                                    start=(ci == 0), stop=(ci == 2))
                            nc.vector.tensor_scalar_mul(
                                out=ot[:, tq * 512:(tq + 1) * 512],
                                in0=ps_o[0:ms, :],
                                scalar1=invs[mi][0:ms, :])
                        nc.sync.dma_start(
                            out=out_d[b, m0:m0 + ms, t0:t0 + TH], in_=ot)

    nc.compile()
    return nc


def _get_nc():
    if "nc" not in _NC_CACHE:
        _NC_CACHE["nc"] = _build_bass()
    return _NC_CACHE["nc"]


def _prep_host(meg, positions, subject_index, heads):
    """Build the 8 per-core input maps from the full inputs."""
    f32 = np.float32
    pos = np.asarray(positions, dtype=f32)
    a = ((pos[:, :, 0] + MARGIN) / WIDTH).astype(f32)           # [B, C]
    bcoord = ((pos[:, :, 1] + MARGIN) / WIDTH).astype(f32)      # [B, C]
    invalid = np.all(pos == INVALID, axis=-1)                   # [B, C]
    offs = np.where(invalid, f32(NEG_INF), f32(0.0)).astype(f32)

    h = np.asarray(heads, dtype=f32)[np.asarray(subject_index).astype(np.int64)]
    hT = np.ascontiguousarray(h.transpose(0, 2, 1))             # [B, 242, O]
    hta = np.concatenate(
        [hT[:, :NF2, :], np.ones((B, 1, CHOUT), dtype=f32)], axis=1)
    htb = np.ascontiguousarray(hT[:, NF2:, :])

    fr = np.arange(N_FREQS, dtype=f32)
    fi = np.repeat(fr, N_FREQS).reshape(NF2, 1)
    fj = np.tile(fr, N_FREQS).reshape(NF2, 1)
    ones = np.ones((128, 1), dtype=f32)

    megf = np.asarray(meg, dtype=f32)
    in_maps = []
    for c in range(N_CORES):
        s = slice(c * BS, (c + 1) * BS)
        in_maps.append(dict(
            meg=np.ascontiguousarray(megf[s]),
            pa=np.ascontiguousarray(a[s]),
            pb=np.ascontiguousarray(bcoord[s]),
            offs=np.ascontiguousarray(offs[s]),
            hta=np.ascontiguousarray(hta[s]),
            htb=np.ascontiguousarray(htb[s]),
            fi=fi, fj=fj, ones=ones,
        ))
    return in_maps


def kernel(meg, positions, subject_index, heads, _trace=False):
    from concourse.bass_utils import run_bass_kernel_spmd

    nc = _get_nc()
    in_maps = _prep_host(meg, positions, subject_index, heads)
    res = run_bass_kernel_spmd(nc, in_maps, core_ids=list(range(N_CORES)),
                               trace=_trace)
    out = np.concatenate([r["out"] for r in res.results], axis=0)
    if _trace:
        kernel.last_exec_time_ns = res.exec_time_ns
        kernel.last_results = res
    return out.astype(np.float32)


# revision 5
# speedup vs baseline: 1.0202x; 1.0202x over previous
"""TRN2 Bass kernel for nn_BrainModule (sparse_attention).

Computation (per sample b):
  emb[c,d]   = fourier embedding of positions[b,c]          (d = 242)
  scores[o,c]= heads[subj[b]][o,:] . emb[c,:] + offset[c]   (offset = -1e9 on
                                                             invalid channels)
  w[o,c]     = softmax_c(scores)
  out[o,t]   = sum_c w[o,c] * meg[b,c,t]

Strategy: data-parallel over batch B=32 across 8 cores (4 samples each).
On device, everything is computed in the [C, O] ("transposed") orientation so
the big einsum consumes the softmax weights directly as the matmul stationary
operand, with the 1/sum normalization folded into the PSUM->SBUF copy.
All matmuls use float32r (1 cycle/row at N>=256 vs 4 for fp32; measured HW
rel-err ~2e-4, identical to the fp32 path on this silicon).

The program is phase-ordered to keep the PE HAM-warm and to avoid ACT
table thrash:
  phase 1a: fourier embeddings for all samples   (DVE chain + ACT Sin)
  phase 1b: scores + exp + row-sums for all samples (PE small MMs + ACT Exp)
  phase 2:  the big matmuls for all samples back-to-back (PE dense)

Channels past the valid prefix contribute exactly 0 weight (exp(-1e9) == 0
in fp32), so when the invalid channels form a suffix (always true for this
module: the last 16 channels are sentinel) the kernel is built for the
shorter channel prefix and skips their meg DMA entirely; otherwise it falls
back to all 273 channels with the -1e9 offset folded into the score matmul
as an extra K row.
"""
import numpy as np

B, C, T = 32, 273, 4096
CHOUT = 270
N_FREQS = 11
NF2 = N_FREQS * N_FREQS          # 121
D_A = NF2 + 1                    # cos half + offset/ones row
MARGIN = 0.2
WIDTH = 1.0 + 2.0 * MARGIN
INVALID = -0.1
NEG_INF = -1e9
N_CORES = 8
BS = B // N_CORES                # samples per core
TWO_PI = float(2.0 * np.pi)
# largest f32 <= 2*pi, so |frac| = 0.5 never maps beyond pi
SCALE_2PI = float(np.nextafter(np.float32(2.0 * np.pi), np.float32(0.0)))

M_CHUNKS = [(0, 128), (128, 128), (256, CHOUT - 256)]  # partition chunks of O
TH = 2048                                              # out-tile t width
NT_Q = TH // 512                                       # 512-wide psum tiles

_NC_CACHE = {}


def _c_chunks(c_used):
    out = []
    c0 = 0
    while c0 < c_used:
        out.append((c0, min(128, c_used - c0)))
        c0 += 128
    return out


def _build_bass(c_used):
    import concourse.bacc as bacc
    import concourse.mybir as mybir
    import concourse.tile as tile
    import concourse.bass as bass

    F32 = mybir.dt.float32
    F32R = mybir.dt.float32r
    I32 = mybir.dt.int32
    Sin = mybir.ActivationFunctionType.Sin
    Exp = mybir.ActivationFunctionType.Exp
    Copy = mybir.ActivationFunctionType.Copy

    CC = _c_chunks(c_used)
    NCC = len(CC)

    nc = bacc.Bacc("TRN2", target_bir_lowering=False, debug=False,
                   num_devices=N_CORES)

    meg_d = nc.dram_tensor("meg", [BS, C, T], F32R, kind="ExternalInput")
    pa_d = nc.dram_tensor("pa", [BS, C], F32, kind="ExternalInput")
    pb_d = nc.dram_tensor("pb", [BS, C], F32, kind="ExternalInput")
    offs_d = nc.dram_tensor("offs", [BS, C], F32R, kind="ExternalInput")
    hta_d = nc.dram_tensor("hta", [BS, D_A, CHOUT], F32R, kind="ExternalInput")
    htb_d = nc.dram_tensor("htb", [BS, NF2, CHOUT], F32R, kind="ExternalInput")
    fi_d = nc.dram_tensor("fi", [NF2, 1], F32, kind="ExternalInput")
    fj_d = nc.dram_tensor("fj", [NF2, 1], F32, kind="ExternalInput")
    ones_d = nc.dram_tensor("ones", [128, 1], F32, kind="ExternalInput")
    out_d = nc.dram_tensor("out", [BS, CHOUT, T], F32, kind="ExternalOutput")

    with tile.TileContext(nc) as tc:
        with (
            tc.tile_pool(name="const", bufs=1) as const,
            tc.tile_pool(name="wsb", bufs=2) as wsb,
            tc.tile_pool(name="persist", bufs=BS) as persist,
            tc.tile_pool(name="megp", bufs=2) as megp,
            tc.tile_pool(name="outp", bufs=2) as outp,
            tc.tile_pool(name="wps", bufs=2, space="PSUM") as wps,
            tc.tile_pool(name="bps", bufs=4, space="PSUM") as bps,
        ):
            fi = const.tile([NF2, 1], F32, tag="fi")
            fj = const.tile([NF2, 1], F32, tag="fj")
            ones = const.tile([128, 1], F32, tag="ones")
            nc.sync.dma_start(out=fi, in_=fi_d[:, :])
            nc.sync.dma_start(out=fj, in_=fj_d[:, :])
            nc.sync.dma_start(out=ones, in_=ones_d[:, :])

            # ---- phase 1a: fourier embeddings (all samples) ------------
            embAs, embBs = [], []
            for b in range(BS):
                a_rep = wsb.tile([NF2, C], F32, tag="a_rep")
                b_rep = wsb.tile([NF2, C], F32, tag="b_rep")
                pa_bcast = bass.AP(
                    tensor=pa_d, offset=b * C, ap=[[0, NF2], [1, C]])
                pb_bcast = bass.AP(
                    tensor=pb_d, offset=b * C, ap=[[0, NF2], [1, C]])
                nc.gpsimd.dma_start(out=a_rep, in_=pa_bcast)
                nc.gpsimd.dma_start(out=b_rep, in_=pb_bcast)

                xs = wsb.tile([NF2, C], F32, tag="xs")
                nc.vector.tensor_scalar_mul(out=xs, in0=a_rep, scalar1=fi)
                xs2 = wsb.tile([NF2, C], F32, tag="xs2")
                nc.vector.tensor_scalar_mul(out=xs2, in0=b_rep, scalar1=fj)
                nc.vector.tensor_add(out=xs, in0=xs, in1=xs2)

                embA = persist.tile([D_A, C], F32R, tag="embA")
                embB = persist.tile([NF2, C], F32R, tag="embB")

                # two-stage range reduction to [-0.5, 0.5]; robust to the
                # f32->int32 cast semantics (HW rounds, CoreSim truncates)
                ki = wsb.tile([NF2, C], I32, tag="ki")
                kf = wsb.tile([NF2, C], F32, tag="kf")
                frac = wsb.tile([NF2, C], F32, tag="frac")

                def reduce_frac(src):
                    nc.vector.tensor_copy(ki, src)
                    nc.vector.tensor_copy(kf, ki)
                    nc.vector.tensor_sub(out=frac, in0=src, in1=kf)
                    nc.vector.tensor_scalar(out=kf, in0=frac, scalar1=0.5,
                                            scalar2=None,
                                            op0=mybir.AluOpType.is_gt)
                    nc.vector.tensor_sub(out=frac, in0=frac, in1=kf)
                    nc.vector.tensor_scalar(out=kf, in0=frac, scalar1=-0.5,
                                            scalar2=None,
                                            op0=mybir.AluOpType.is_lt)
                    nc.vector.tensor_add(out=frac, in0=frac, in1=kf)

                reduce_frac(xs)
                nc.scalar.activation(out=embB, in_=frac, func=Sin,
                                     scale=SCALE_2PI)
                # cos half: cos(2pi x) = sin(2pi (x + 0.25))
                nc.vector.tensor_scalar_add(out=xs2, in0=xs, scalar1=0.25)
                reduce_frac(xs2)
                nc.scalar.activation(out=embA[0:NF2, :], in_=frac, func=Sin,
                                     scale=SCALE_2PI)
                nc.sync.dma_start(out=embA[NF2:D_A, :], in_=offs_d[b:b + 1, :])
                embAs.append(embA)
                embBs.append(embB)

            # ---- phase 1b: scores + exp + sums (all samples) -----------
            wts, invss = [], []
            for b in range(BS):
                hta = wsb.tile([D_A, CHOUT], F32R, tag="hta")
                htb = wsb.tile([NF2, CHOUT], F32R, tag="htb")
                nc.sync.dma_start(out=hta, in_=hta_d[b, :, :])
                nc.sync.dma_start(out=htb, in_=htb_d[b, :, :])
                embA, embB = embAs[b], embBs[b]

                wt = []
                for ci, (c0, cs) in enumerate(CC):
                    ps_s = wps.tile([128, CHOUT], F32, tag="ps_s")
                    nc.tensor.matmul(ps_s[0:cs, :], embA[:, c0:c0 + cs], hta,
                                     start=True, stop=False)
                    nc.tensor.matmul(ps_s[0:cs, :], embB[:, c0:c0 + cs], htb,
                                     start=False, stop=True)
                    w_un = persist.tile([128, CHOUT], F32R, tag=f"w_un{ci}")
                    nc.scalar.activation(out=w_un[0:cs, :], in_=ps_s[0:cs, :],
                                         func=Exp)
                    wt.append(w_un)

                invs = []
                for mi, (m0, ms) in enumerate(M_CHUNKS):
                    ps_sum = wps.tile([128, 1], F32, tag="ps_sum")
                    for ci, (c0, cs) in enumerate(CC):
                        nc.tensor.matmul(ps_sum[0:ms, :],
                                         wt[ci][0:cs, m0:m0 + ms].bitcast(F32),
                                         ones[0:cs, :],
                                         start=(ci == 0), stop=(ci == NCC - 1))
                    inv = persist.tile([128, 1], F32, tag=f"inv{mi}")
                    nc.vector.reciprocal(out=inv[0:ms, :], in_=ps_sum[0:ms, :])
                    invs.append(inv)
                wts.append(wt)
                invss.append(invs)

            # ---- phase 2: big matmuls, PE back-to-back -----------------
            for b in range(BS):
                wt, invs = wts[b], invss[b]
                megs = []
                for ci, (c0, cs) in enumerate(CC):
                    mg = megp.tile([cs, T], F32R, tag=f"mg{ci}")
                    nc.sync.dma_start(out=mg, in_=meg_d[b, c0:c0 + cs, :])
                    megs.append(mg)
                for th in range(T // TH):
                    t0 = th * TH
                    for mi, (m0, ms) in enumerate(M_CHUNKS):
                        ot = outp.tile([ms, TH], F32, tag=f"ot{mi}")
                        for tq in range(NT_Q):
                            ps_o = bps.tile([128, 512], F32, tag="ps_o")
                            for ci, (c0, cs) in enumerate(CC):
                                nc.tensor.matmul(
                                    ps_o[0:ms, :],
                                    wt[ci][0:cs, m0:m0 + ms],
                                    megs[ci][:, t0 + tq * 512:
                                             t0 + (tq + 1) * 512],
                                    start=(ci == 0), stop=(ci == NCC - 1))
                            # scaled psum->sbuf copy; alternate DVE/ACT so
                            # neither engine becomes the bottleneck
                            if tq % 2 == 0:
                                nc.vector.tensor_scalar_mul(
                                    out=ot[:, tq * 512:(tq + 1) * 512],
                                    in0=ps_o[0:ms, :],
                                    scalar1=invs[mi][0:ms, :])
                            else:
                                nc.scalar.activation(
                                    out=ot[:, tq * 512:(tq + 1) * 512],
                                    in_=ps_o[0:ms, :], func=Copy,
                                    scale=invs[mi][0:ms, :])
                        nc.sync.dma_start(
                            out=out_d[b, m0:m0 + ms, t0:t0 + TH], in_=ot)

    nc.compile()
    return nc


def _get_nc(c_used):
    if c_used not in _NC_CACHE:
        _NC_CACHE[c_used] = _build_bass(c_used)
    return _NC_CACHE[c_used]


def _prep_host(meg, positions, subject_index, heads):
    """Build the 8 per-core input maps + pick the channel prefix length."""
    f32 = np.float32
    pos = np.asarray(positions, dtype=f32)
    a = ((pos[:, :, 0] + MARGIN) / WIDTH).astype(f32)           # [B, C]
    bcoord = ((pos[:, :, 1] + MARGIN) / WIDTH).astype(f32)      # [B, C]
    invalid = np.all(pos == INVALID, axis=-1)                   # [B, C]
    offs = np.where(invalid, f32(NEG_INF), f32(0.0)).astype(f32)

    # channels invalid in EVERY sample get weight exactly 0 (exp(-1e9)==0)
    # -> their meg data is never needed; use the valid prefix length
    valid_any = ~np.all(invalid, axis=0)                        # [C]
    c_used = int(np.max(np.nonzero(valid_any)[0])) + 1 if valid_any.any() else C

    h = np.asarray(heads, dtype=f32)[np.asarray(subject_index).astype(np.int64)]
    hT = np.ascontiguousarray(h.transpose(0, 2, 1))             # [B, 242, O]
    hta = np.concatenate(
        [hT[:, :NF2, :], np.ones((B, 1, CHOUT), dtype=f32)], axis=1)
    htb = np.ascontiguousarray(hT[:, NF2:, :])

    fr = np.arange(N_FREQS, dtype=f32)
    fi = np.repeat(fr, N_FREQS).reshape(NF2, 1)
    fj = np.tile(fr, N_FREQS).reshape(NF2, 1)
    ones = np.ones((128, 1), dtype=f32)

    megf = np.asarray(meg, dtype=f32)
    in_maps = []
    for c in range(N_CORES):
        s = slice(c * BS, (c + 1) * BS)
        in_maps.append(dict(
            meg=np.ascontiguousarray(megf[s]),
            pa=np.ascontiguousarray(a[s]),
            pb=np.ascontiguousarray(bcoord[s]),
            offs=np.ascontiguousarray(offs[s]),
            hta=np.ascontiguousarray(hta[s]),
            htb=np.ascontiguousarray(htb[s]),
            fi=fi, fj=fj, ones=ones,
        ))
    return in_maps, c_used


def kernel(meg, positions, subject_index, heads, _trace=False):
    from concourse.bass_utils import run_bass_kernel_spmd

    in_maps, c_used = _prep_host(meg, positions, subject_index, heads)
    nc = _get_nc(c_used)
    res = run_bass_kernel_spmd(nc, in_maps, core_ids=list(range(N_CORES)),
                               trace=_trace)
    out = np.concatenate([r["out"] for r in res.results], axis=0)
    if _trace:
        kernel.last_exec_time_ns = res.exec_time_ns
        kernel.last_results = res
    return out.astype(np.float32)
